# revision 13
# baseline (speedup 1.0000x reference)
"""Trainium2 Bass kernel for windowed/global sparse attention (Swin-style
relative-position bias + 1 global token), data-parallel over batch on 8 cores.

Shapes: B=16, N=785 (1 global + 28x28 local), C=768, H=12 heads, d=64.

Key perf facts (measured on HW):
  - The PE clock runs at 2.4 GHz only when the stationary operand spans the
    full 128 contraction rows; <=64-row matmuls are clocked at 1.2 GHz.
    Zero-padding the stationary tile to 128 rows restores 2.4 GHz, so kT is
    stored per-head as [128, N] with the other head-pair half zeroed (the
    rhs qT tile carries both heads; zero weight rows kill the wrong head).
    The same trick pads the 17-key tail chunk of the O matmul to 128 rows.
  - Matmul cost is out-free-size cycles; contraction row count is free.
    fp8 DoubleRow only pays at K>128 and fails the accuracy budget here,
    so everything stays bf16.

Per-core device program (2 batches/core, software-pipelined as in the
original design: attention S/exp/mult for pair j overlaps the O convoy for
pair j-1, with the other batch's QKV and the previous batch's projection
emitted as PE gap-filler):
  - qT computed transposed ([d, tokens]); kT per-head zero-padded [128, N].
  - softmax: exp(S) * expB with expB = exp(bias) precomputed on host.
  - V carries a ones column per head so the O matmul also yields softmax
    denominators; denominator rows are extracted on GpSimd, batch-reciprocal
    on DVE, DMA-broadcast back, multiplied into O^T in place.
  - projection is emitted transposed (out^T[co, n] accumulated over ci with
    pwT stationary); the ACT engine fuses the bias add via Identity
    activation with a per-partition bias vector; the host un-transposes.
"""

import numpy as np
import ml_dtypes

import concourse.bass as bass
import concourse.bacc as bacc
import concourse.tile as tile
from concourse.tile import add_dep_helper
from concourse import mybir
from concourse.bass_utils import run_bass_kernel_spmd

F32 = mybir.dt.float32
BF16 = mybir.dt.bfloat16

WX = WY = 28
NGLO = 1
H = 12
L = WX * WY            # 784
N = NGLO + L           # 785
C = 768
HD = C // H            # 64
SCALE = HD ** -0.5
B = 16
N_CORES = 8
B_LOC = B // N_CORES   # 2
NCC = C // 128         # 6 contraction chunks
NKC = (N + 127) // 128  # 7 key/token chunks (last = 17 rows)
W = 786                # padded free width for N-sized tiles (even, 4B-aligned)

CG_N = [(0, 512), (512, 274)]
CG_C = [(0, 512), (512, 256)]


def _kr(kc):
    return min(128, N - kc * 128)


def build_nc():
    nc = bacc.Bacc(None, target_bir_lowering=False)

    xT_d = nc.dram_tensor("xT", [B_LOC, C, W], BF16, kind="ExternalInput")
    qkvwT_d = nc.dram_tensor("qkv_wT", [C, 3 * C], BF16, kind="ExternalInput")
    pwT_d = nc.dram_tensor("proj_wT", [C, C], BF16, kind="ExternalInput")
    pb_d = nc.dram_tensor("proj_b", [128, NCC], F32, kind="ExternalInput")
    expB_d = nc.dram_tensor("expB", [H, N, N], BF16, kind="ExternalInput")
    out_d = nc.dram_tensor("out", [B_LOC, C, N], F32, kind="ExternalOutput")
    dinv_d = nc.dram_tensor("dinv_scratch", [B_LOC, H, N], BF16)

    with tile.TileContext(nc) as tc:
        with (
            tc.tile_pool(name="consts", bufs=1) as consts,
            tc.tile_pool(name="perb", bufs=2) as perb,
            tc.tile_pool(name="expbp", bufs=5) as expbp,
            tc.tile_pool(name="flow", bufs=4) as flow,
            tc.tile_pool(name="ptp", bufs=16) as ptp,
            tc.tile_pool(name="norm", bufs=1) as norm,
            tc.tile_pool(name="outp", bufs=2) as outp,
            tc.tile_pool(name="psum_s", bufs=4, space=bass.MemorySpace.PSUM) as psum_s,
        ):
            # ---- weights (resident, bf16); proj weights loaded last ----
            qkvw = []
            for cc in range(NCC):
                t = consts.tile([128, 3 * C], BF16, tag=f"qkvw{cc}", name=f"qkvw{cc}")
                qkvw.append(t)
            pw16 = []
            for cc in range(NCC):
                t = consts.tile([128, C], BF16, tag=f"pw{cc}", name=f"pw{cc}")
                pw16.append(t)
            pb_sb = consts.tile([128, NCC], F32, tag="pbsb")

            def emit_weight_loads_qkv():
                for cc in range(NCC):
                    nc.sync.dma_start(
                        qkvw[cc][:], qkvwT_d[cc * 128:(cc + 1) * 128, :]
                    )

            def emit_weight_loads_proj():
                for cc in range(NCC):
                    nc.sync.dma_start(
                        pw16[cc][:], pwT_d[cc * 128:(cc + 1) * 128, :]
                    )
                nc.sync.dma_start(pb_sb[:], pb_d[:])

            def emit_x(b):
                xts = []
                for cc in range(NCC):
                    t = perb.tile([128, W], BF16, tag=f"xt{cc}", name=f"xt{cc}_{b}")
                    nc.sync.dma_start(t[:], xT_d[b, cc * 128:(cc + 1) * 128, :])
                    xts.append(t)
                return xts

            def alloc_kTz(b):
                """Per-head zero-padded kT tiles [128, W]: head 2j at rows
                0:64 (rows 64:128 zero), head 2j+1 at rows 64:128."""
                kTz = [perb.tile([128, W], BF16, tag=f"kTz{h}", name=f"kTz{h}_{b}")
                       for h in range(H)]
                for j in range(NCC):
                    nc.gpsimd.memset(kTz[2 * j][64:128, :], 0.0)
                    nc.gpsimd.memset(kTz[2 * j + 1][0:64, :], 0.0)
                return kTz

            def emit_qkvT_chunk(b, xts, j, qT, kTz, evac_vector):
                """produce qT[j] (both heads stacked) and kTz[2j], kTz[2j+1]."""
                firsts = []
                for oc in (j, NCC + j):
                    ps = psum_s.tile([128, W], F32, tag="s", name=f"psqk{oc}_{b}")
                    for cc in range(NCC):
                        for (c0, cn) in CG_N:
                            mm = nc.tensor.matmul(
                                ps[:, c0:c0 + cn],
                                qkvw[cc][:, oc * 128:(oc + 1) * 128],
                                xts[cc][:, c0:c0 + cn],
                                start=(cc == 0),
                                stop=(cc == NCC - 1),
                            )
                            if cc == 0 and c0 == 0:
                                firsts.append(mm)
                    if oc < NCC:
                        dst = qT[oc]
                        if evac_vector:
                            nc.vector.tensor_copy(dst[:, 0:N], ps[:, 0:N])
                        else:
                            nc.scalar.copy(dst[:, 0:N], ps[:, 0:N])
                        nc.vector.memset(dst[:, N:W], 0.0)
                    else:
                        h = 2 * j
                        # key columns beyond N-1 are never read by the S
                        # matmuls (lhsT slices stop at key 784), so only the
                        # 0:N region needs real values. gpsimd cannot read
                        # PSUM, so these ride the same engine split as qT.
                        nc.vector.tensor_copy(kTz[h][0:64, 0:N], ps[0:64, 0:N])
                        nc.vector.tensor_copy(
                            kTz[h + 1][64:128, 0:N], ps[64:128, 0:N])
                return firsts

            def emit_v(b, xts):
                vp = [perb.tile([128, H * (HD + 1)], BF16, tag=f"vp{i}",
                                name=f"vp{i}_{b}") for i in range(NKC)]
                vfirsts = []
                for kc in range(NKC):
                    kr = _kr(kc)
                    ps = psum_s.tile([128, C], F32, tag="s", name=f"psv{kc}_{b}")
                    for cc in range(NCC):
                        for (c0, cn) in CG_C:
                            mm = nc.tensor.matmul(
                                ps[0:kr, c0:c0 + cn],
                                xts[cc][:, kc * 128:kc * 128 + kr],
                                qkvw[cc][:, 2 * C + c0:2 * C + c0 + cn],
                                start=(cc == 0),
                                stop=(cc == NCC - 1),
                            )
                            if cc == 0 and c0 == 0:
                                vfirsts.append(mm)
                    if kr < 128:
                        # zero the whole tile first (gpsimd memsets must be
                        # partition-aligned); the copies below fill 0:kr and
                        # rows kr:128 stay zero so the O matmul can stream
                        # the full 128-row stationary tile at 2.4 GHz
                        nc.gpsimd.memset(vp[kc][:], 0.0)
                    v3 = vp[kc][:].rearrange("p (h e) -> p h e", e=HD + 1)
                    nc.vector.tensor_copy(
                        v3[0:kr, :, 0:HD],
                        ps[0:kr, :].rearrange("p (h d) -> p h d", d=HD),
                    )
                    nc.vector.memset(v3[0:kr, :, HD:HD + 1], 1.0)
                return vp, vfirsts

            def alloc_oT(b):
                return [perb.tile([128, W], BF16, tag=f"oT{i}", name=f"oT{i}_{b}")
                        for i in range(NCC)]

            def emit_attn_pass1(b, j, qT, kTz):
                """S + exp + expB-multiply for head pair (2j, 2j+1).
                S matmuls use the full-128-row zero-padded kTz stationary so
                the PE runs at 2.4 GHz. Returns P tiles and a pacer matmul."""
                pts = [[None, None] for _ in range(NKC)]
                pacer = None
                for kc in range(NKC):
                    kr = _kr(kc)
                    ps_ss = [
                        psum_s.tile([128, W], F32, tag="s",
                                    name=f"pss{2 * j + hh}_{kc}_{b}")
                        for hh in range(2)
                    ]
                    for (c0, cn) in CG_N:
                        for hh in range(2):
                            h = 2 * j + hh
                            mm = nc.tensor.matmul(
                                ps_ss[hh][0:kr, c0:c0 + cn],
                                kTz[h][:, kc * 128:kc * 128 + kr],
                                qT[j][:, c0:c0 + cn],
                                start=True,
                                stop=True,
                            )
                            if kc == 2 and pacer is None:
                                pacer = mm
                    for hh in range(2):
                        h = 2 * j + hh
                        ebt = expbp.tile([128, W], BF16, tag="expb",
                                         name=f"ebt{h}_{kc}_{b}")
                        nc.vector.memset(ebt[:, N:W], 0.0)
                        nc.sync.dma_start(
                            ebt[0:kr, 0:N],
                            expB_d[h, kc * 128:kc * 128 + kr, :],
                        )
                        es = flow.tile([128, W], BF16, tag="expS",
                                       name=f"es{h}_{kc}_{b}")
                        nc.scalar.activation(
                            es[0:kr, 0:W], ps_ss[hh][0:kr, 0:W],
                            mybir.ActivationFunctionType.Exp,
                        )
                        pt = ptp.tile([128, W], BF16, tag="pT",
                                      name=f"pt{h}_{kc}_{b}")
                        if kr < 128:
                            # rows kr:128 must be zero (not stale garbage):
                            # the O matmul streams the full 128 rows and
                            # Inf/NaN garbage would poison psum even against
                            # zero V rows
                            nc.gpsimd.memset(pt[:], 0.0)
                        nc.vector.tensor_tensor(
                            pt[0:kr, 0:W],
                            es[0:kr, 0:W],
                            ebt[0:kr, 0:W],
                            mybir.AluOpType.mult,
                        )
                        pts[kc][hh] = pt
                return pts, pacer

            def emit_attn_pass2(b, j, pts, vp, oT, dallp):
                """dense O-accumulation convoy for head pair (2j, 2j+1)."""
                for hh in range(2):
                    h = 2 * j + hh
                    ps_o = psum_s.tile([HD + 1, W], F32, tag="s",
                                       name=f"pso{h}_{b}")
                    for kc in range(NKC):
                        for (c0, cn) in CG_N:
                            nc.tensor.matmul(
                                ps_o[:, c0:c0 + cn],
                                vp[kc][0:128, h * (HD + 1):(h + 1) * (HD + 1)],
                                pts[kc][hh][0:128, c0:c0 + cn],
                                start=(kc == 0),
                                stop=(kc == NKC - 1),
                            )
                    nc.vector.tensor_copy(
                        oT[j][hh * 64:hh * 64 + 64, 0:N], ps_o[0:64, 0:N]
                    )
                    dn = norm.tile([65, W], BF16, tag="dn", bufs=1,
                                   name=f"dn{h}_{b}")
                    nc.vector.tensor_copy(dn[64:65, 0:N], ps_o[64:65, 0:N])
                    nc.sync.dma_start(dallp[hh:hh + 1, 0:N], dn[64:65, 0:N])

            def emit_norm_pair(b, j, oT, dallp):
                # incremental: reciprocal + broadcast + normalize for one
                # head pair as soon as its O convoy lands, so the tail only
                # waits on the final pair instead of the whole batch
                dinvp = norm.tile([2, W], BF16, tag="dinvp", bufs=2,
                                  name=f"dinvp{j}_{b}")
                with nc.allow_low_precision(
                        reason="bf16 softmax denominators: 0.1% rms, within budget"):
                    nc.vector.reciprocal(dinvp[0:2, 0:N], dallp[0:2, 0:N])
                nc.sync.dma_start(dinv_d[b, 2 * j:2 * j + 2], dinvp[0:2, 0:N])
                dr = norm.tile([128, W], BF16, tag="drep", bufs=2,
                               name=f"dr{j}_{b}")
                for hh in range(2):
                    row = dinv_d[b, 2 * j + hh, :]
                    srow = bass.AP(
                        tensor=row.tensor, offset=row.offset,
                        ap=[[0, 64]] + row.ap,
                    )
                    nc.sync.dma_start(dr[hh * 64:(hh + 1) * 64, 0:N], srow)
                nc.vector.tensor_tensor(
                    oT[j][:, 0:N], oT[j][:, 0:N], dr[:, 0:N],
                    mybir.AluOpType.mult,
                )

            def emit_proj(b, oT):
                """transposed projection: out^T[co, n] = sum_ci pwT.T oT,
                bias fused on ACT via Identity activation."""
                pfirsts = []
                for co in range(NCC):
                    ps = psum_s.tile([128, W], F32, tag="s", name=f"psp{co}_{b}")
                    for cc in range(NCC):
                        for (c0, cn) in CG_N:
                            mm = nc.tensor.matmul(
                                ps[:, c0:c0 + cn],
                                pw16[cc][:, co * 128:(co + 1) * 128],
                                oT[cc][:, c0:c0 + cn],
                                start=(cc == 0),
                                stop=(cc == NCC - 1),
                            )
                            if cc == 0 and c0 == 0:
                                pfirsts.append(mm)
                    ob = outp.tile([128, W], F32, tag="ob", name=f"ob{co}_{b}")
                    nc.scalar.activation(
                        ob[:, 0:N], ps[:, 0:N],
                        mybir.ActivationFunctionType.Identity,
                        bias=pb_sb[:, co:co + 1],
                    )
                    nc.sync.dma_start(
                        out_d[b, co * 128:(co + 1) * 128, :], ob[:, 0:N]
                    )
                return pfirsts

            # software pipeline: batch 1's QKV work is emitted at lower
            # priority than batch 0's attention (and just-in-time between
            # batch 1's attention pairs) so the Tile scheduler uses it as PE
            # gap-filler during the exp/multiply-paced attention phases.
            # interleave weight/x chunk loads so the j=0 convoy's cc=0
            # matmul can start after the first two transfers instead of
            # waiting out the whole load train
            xts0 = []
            for cc in range(NCC):
                nc.sync.dma_start(
                    qkvw[cc][:], qkvwT_d[cc * 128:(cc + 1) * 128, :]
                )
                t = perb.tile([128, W], BF16, tag=f"xt{cc}", name=f"xt{cc}_0")
                nc.sync.dma_start(t[:], xT_d[0, cc * 128:(cc + 1) * 128, :])
                xts0.append(t)
            qT0 = [perb.tile([128, W], BF16, tag=f"qT{i}", name=f"qT{i}_0")
                   for i in range(NCC)]
            kTz0 = alloc_kTz(0)
            for j in range(NCC):
                emit_qkvT_chunk(0, xts0, j, qT0, kTz0, evac_vector=False)
            vp0, _ = emit_v(0, xts0)
            emit_weight_loads_proj()

            oT0 = alloc_oT(0)
            pacers0 = []
            pend0 = []

            def dallp_tile(b, j):
                return norm.tile([2, W], BF16, tag="dallp", bufs=2,
                                 name=f"dallp{j}_{b}")

            for j in range(NCC):
                if j >= 1:
                    dp = dallp_tile(0, j - 1)
                    emit_attn_pass2(0, j - 1, pend0[j - 1], vp0, oT0, dp)
                    emit_norm_pair(0, j - 1, oT0, dp)
                pts_j, pac = emit_attn_pass1(0, j, qT0, kTz0)
                pacers0.append(pac)
                pend0.append(pts_j)
            dp = dallp_tile(0, NCC - 1)
            emit_attn_pass2(0, NCC - 1, pend0[NCC - 1], vp0, oT0, dp)
            emit_norm_pair(0, NCC - 1, oT0, dp)
            # fillers for batch-0 attention: x1 load + V1 + qkvT1, paced so
            # the greedy scheduler doesn't front-load them all at once
            xts1 = emit_x(1)
            vp1, vfirsts1 = emit_v(1, xts1)
            for kc, f in enumerate(vfirsts1):
                add_dep_helper(f.ins, pacers0[min(kc, NCC - 1)].ins, sync=False,
                               reason="pace v1 filler")

            qT1 = [perb.tile([128, W], BF16, tag=f"qT{i}", name=f"qT{i}_1")
                   for i in range(NCC)]
            kTz1 = alloc_kTz(1)
            oT1 = alloc_oT(1)
            pacers1 = []
            pend1 = []
            for j in range(NCC):
                # just-in-time qkv chunk for pair j, used as gap-filler.
                qf = emit_qkvT_chunk(1, xts1, j, qT1, kTz1, evac_vector=True)
                pace = pacers0[j + 4] if j < 2 else pacers1[j - 2]
                for f in qf:
                    add_dep_helper(f.ins, pace.ins, sync=False,
                                   reason="pace qkvT1 filler")
                if j >= 1:
                    dp = dallp_tile(1, j - 1)
                    emit_attn_pass2(1, j - 1, pend1[j - 1], vp1, oT1, dp)
                    emit_norm_pair(1, j - 1, oT1, dp)
                pts_j, pac = emit_attn_pass1(1, j, qT1, kTz1)
                pacers1.append(pac)
                pend1.append(pts_j)
            dp = dallp_tile(1, NCC - 1)
            emit_attn_pass2(1, NCC - 1, pend1[NCC - 1], vp1, oT1, dp)
            emit_norm_pair(1, NCC - 1, oT1, dp)
            # proj0: paced across batch-1 attention pairs as its PE filler
            pfirsts0 = emit_proj(0, oT0)
            for tt, f in enumerate(pfirsts0):
                add_dep_helper(f.ins, pacers1[min(tt, NCC - 1)].ins, sync=False,
                               reason="pace proj0 filler")
            emit_proj(1, oT1)

    nc.compile()
    return nc


def _relative_position_index():
    coords = np.stack(np.meshgrid(np.arange(WX), np.arange(WY), indexing="ij"))
    cf = coords.reshape(2, -1)
    rel = cf[:, :, None] - cf[:, None, :]
    rel = rel.transpose(1, 2, 0).astype(np.int64)
    rel[:, :, 0] += WX - 1
    rel[:, :, 1] += WY - 1
    rel[:, :, 0] *= 2 * WY - 1
    return rel.sum(-1)  # [L, L]


def _host_prep(x, qkv_w, proj_w, proj_b, rel_table, g2l, g2g):
    x = np.asarray(x, np.float32)
    qkv_w = np.asarray(qkv_w, np.float32)
    proj_w = np.asarray(proj_w, np.float32)
    proj_b = np.asarray(proj_b, np.float32)
    rel_table = np.asarray(rel_table, np.float32)
    g2l = np.asarray(g2l, np.float32)
    g2g = np.asarray(g2g, np.float32)

    bf16 = ml_dtypes.bfloat16
    xT = np.zeros((B, C, W), np.float32)
    xT[:, :, :N] = x.transpose(0, 2, 1)
    xT = xT.astype(bf16)
    qkv_wT = np.ascontiguousarray(qkv_w.T).copy()                  # [C, 3C]
    qkv_wT[:, :C] *= SCALE                                         # fold q scale
    qkv_wT = qkv_wT.astype(bf16)
    proj_wT = np.ascontiguousarray(proj_w.T).astype(bf16)          # [C, C]
    pb = np.ascontiguousarray(proj_b.reshape(NCC, 128).T)          # [128, NCC]

    # expB[h, k, q] = exp(bias[h, q, k]); exp applied at table granularity,
    # then expanded by the constant-index relative-position gather.
    ridx = _relative_position_index()
    et = np.exp(rel_table)                                         # [3025, H]
    eg2l = np.exp(g2l)                                             # [2, H, 1]
    eg2g = np.exp(g2g)                                             # [H, 1, 1]
    expB = np.empty((H, N, N), np.float32)
    expB[:, 1:, 1:] = et[ridx].transpose(2, 1, 0)                  # [H, k, q]
    expB[:, 0, 0] = eg2g[:, 0, 0]
    expB[:, 1:, 0] = eg2l[0][:, 0][None, :].T                      # global query
    expB[:, 0, 1:] = eg2l[1][:, 0][:, None]                        # global key
    expB16 = expB.astype(bf16)

    in_maps = []
    for i in range(N_CORES):
        in_maps.append({
            "xT": xT[i * B_LOC:(i + 1) * B_LOC],
            "qkv_wT": qkv_wT,
            "proj_wT": proj_wT,
            "proj_b": pb,
            "expB": expB16,
        })
    return in_maps


_NC = None


def get_nc():
    global _NC
    if _NC is None:
        _NC = build_nc()
    return _NC


def kernel(x, qkv_w, proj_w, proj_b, rel_table, g2l, g2g):
    in_maps = _host_prep(x, qkv_w, proj_w, proj_b, rel_table, g2l, g2g)
    nc = get_nc()
    res = run_bass_kernel_spmd(nc, in_maps, core_ids=list(range(N_CORES)))
    # device emits out^T [B_LOC, C, N]; un-transpose on host
    out = np.concatenate(
        [res.results[i]["out"].transpose(0, 2, 1) for i in range(N_CORES)],
        axis=0,
    )
    return np.ascontiguousarray(out).astype(np.float32)


# revision 14
# speedup vs baseline: 1.0755x; 1.0755x over previous
"""Trainium2 Bass kernel for windowed/global sparse attention (Swin-style
relative-position bias + 1 global token), data-parallel over batch on 8 cores.

Shapes: B=16, N=785 (1 global + 28x28 local), C=768, H=12 heads, d=64.

Key perf facts (measured on HW):
  - The PE clock runs at 2.4 GHz only when the stationary operand spans the
    full 128 contraction rows; <=64-row matmuls are clocked at 1.2 GHz.
    Zero-padding the stationary tile to 128 rows restores 2.4 GHz, so kT is
    stored per-head as [128, N] with the other head-pair half zeroed (the
    rhs qT tile carries both heads; zero weight rows kill the wrong head).
    The same trick pads the 17-key tail chunk of the O matmul to 128 rows.
  - Matmul cost is out-free-size cycles; contraction row count is free.
    fp8 DoubleRow only pays at K>128 and fails the accuracy budget here,
    so everything stays bf16.

Per-core device program (2 batches/core, software-pipelined as in the
original design: attention S/exp/mult for pair j overlaps the O convoy for
pair j-1, with the other batch's QKV and the previous batch's projection
emitted as PE gap-filler):
  - qT computed transposed ([d, tokens]); kT per-head zero-padded [128, N].
  - softmax: exp(S) * expB with expB = exp(bias) precomputed on host.
  - V carries a ones column per head so the O matmul also yields softmax
    denominators; denominator rows are extracted on GpSimd, batch-reciprocal
    on DVE, DMA-broadcast back, multiplied into O^T in place.
  - projection is emitted transposed (out^T[co, n] accumulated over ci with
    pwT stationary); the ACT engine fuses the bias add via Identity
    activation with a per-partition bias vector; the host un-transposes.
"""

import numpy as np
import ml_dtypes

import concourse.bass as bass
import concourse.bacc as bacc
import concourse.tile as tile
from concourse.tile import add_dep_helper
from concourse import mybir
from concourse.bass_utils import run_bass_kernel_spmd

F32 = mybir.dt.float32
BF16 = mybir.dt.bfloat16

WX = WY = 28
NGLO = 1
H = 12
L = WX * WY            # 784
N = NGLO + L           # 785
C = 768
HD = C // H            # 64
SCALE = HD ** -0.5
B = 16
N_CORES = 8
B_LOC = B // N_CORES   # 2
NCC = C // 128         # 6 contraction chunks
NKC = (N + 127) // 128  # 7 key/token chunks (last = 17 rows)
W = 786                # padded free width for N-sized tiles (even, 4B-aligned)

CG_N = [(0, 512), (512, 274)]
CG_C = [(0, 512), (512, 256)]


def _kr(kc):
    return min(128, N - kc * 128)


def build_nc():
    nc = bacc.Bacc(None, target_bir_lowering=False)

    xT_d = nc.dram_tensor("xT", [B_LOC, C, W], BF16, kind="ExternalInput")
    qkvwT_d = nc.dram_tensor("qkv_wT", [C, 3 * C], BF16, kind="ExternalInput")
    pwT_d = nc.dram_tensor("proj_wT", [C, C], BF16, kind="ExternalInput")
    pb_d = nc.dram_tensor("proj_b", [128, NCC], F32, kind="ExternalInput")
    expB_d = nc.dram_tensor("expB", [H, N, N], BF16, kind="ExternalInput")
    out_d = nc.dram_tensor("out", [B_LOC, C, N], F32, kind="ExternalOutput")
    dinv_d = nc.dram_tensor("dinv_scratch", [B_LOC, H, N], BF16)

    with tile.TileContext(nc) as tc:
        with (
            tc.tile_pool(name="consts", bufs=1) as consts,
            tc.tile_pool(name="perb", bufs=2) as perb,
            tc.tile_pool(name="expbp", bufs=5) as expbp,
            tc.tile_pool(name="flow", bufs=4) as flow,
            tc.tile_pool(name="ptp", bufs=18) as ptp,
            tc.tile_pool(name="norm", bufs=1) as norm,
            tc.tile_pool(name="outp", bufs=2) as outp,
            tc.tile_pool(name="psum_s", bufs=4, space=bass.MemorySpace.PSUM) as psum_s,
        ):
            # ---- weights (resident, bf16); proj weights loaded last ----
            qkvw = []
            for cc in range(NCC):
                t = consts.tile([128, 3 * C], BF16, tag=f"qkvw{cc}", name=f"qkvw{cc}")
                qkvw.append(t)
            pw16 = []
            for cc in range(NCC):
                t = consts.tile([128, C], BF16, tag=f"pw{cc}", name=f"pw{cc}")
                pw16.append(t)
            pb_sb = consts.tile([128, NCC], F32, tag="pbsb")

            def emit_weight_loads_qkv():
                for cc in range(NCC):
                    nc.sync.dma_start(
                        qkvw[cc][:], qkvwT_d[cc * 128:(cc + 1) * 128, :]
                    )

            def emit_weight_loads_proj():
                for cc in range(NCC):
                    nc.sync.dma_start(
                        pw16[cc][:], pwT_d[cc * 128:(cc + 1) * 128, :]
                    )
                nc.sync.dma_start(pb_sb[:], pb_d[:])

            def emit_x(b):
                xts = []
                for cc in range(NCC):
                    t = perb.tile([128, W], BF16, tag=f"xt{cc}", name=f"xt{cc}_{b}")
                    nc.sync.dma_start(t[:], xT_d[b, cc * 128:(cc + 1) * 128, :])
                    xts.append(t)
                return xts

            def alloc_kTz(b):
                """Per-head zero-padded kT tiles [128, W]: head 2j at rows
                0:64 (rows 64:128 zero), head 2j+1 at rows 64:128."""
                kTz = [perb.tile([128, W], BF16, tag=f"kTz{h}", name=f"kTz{h}_{b}")
                       for h in range(H)]
                for j in range(NCC):
                    nc.gpsimd.memset(kTz[2 * j][64:128, :], 0.0)
                    nc.gpsimd.memset(kTz[2 * j + 1][0:64, :], 0.0)
                return kTz

            def emit_qkvT_chunk(b, xts, j, qT, kTz, evac_vector):
                """produce qT[j] (both heads stacked) and kTz[2j], kTz[2j+1]."""
                firsts = []
                for oc in (j, NCC + j):
                    ps = psum_s.tile([128, W], F32, tag="s", name=f"psqk{oc}_{b}")
                    for cc in range(NCC):
                        for (c0, cn) in CG_N:
                            mm = nc.tensor.matmul(
                                ps[:, c0:c0 + cn],
                                qkvw[cc][:, oc * 128:(oc + 1) * 128],
                                xts[cc][:, c0:c0 + cn],
                                start=(cc == 0),
                                stop=(cc == NCC - 1),
                            )
                            if cc == 0 and c0 == 0:
                                firsts.append(mm)
                    if oc < NCC:
                        dst = qT[oc]
                        if evac_vector:
                            nc.vector.tensor_copy(dst[:, 0:N], ps[:, 0:N])
                        else:
                            nc.scalar.copy(dst[:, 0:N], ps[:, 0:N])
                        nc.vector.memset(dst[:, N:W], 0.0)
                    else:
                        h = 2 * j
                        # key columns beyond N-1 are never read by the S
                        # matmuls (lhsT slices stop at key 784), so only the
                        # 0:N region needs real values. gpsimd cannot read
                        # PSUM, so these ride the same engine split as qT.
                        nc.vector.tensor_copy(kTz[h][0:64, 0:N], ps[0:64, 0:N])
                        nc.vector.tensor_copy(
                            kTz[h + 1][64:128, 0:N], ps[64:128, 0:N])
                return firsts

            def emit_v(b, xts):
                vp = [perb.tile([128, H * (HD + 1)], BF16, tag=f"vp{i}",
                                name=f"vp{i}_{b}") for i in range(NKC)]
                vfirsts = []
                for kc in range(NKC):
                    kr = _kr(kc)
                    ps = psum_s.tile([128, C], F32, tag="s", name=f"psv{kc}_{b}")
                    for cc in range(NCC):
                        for (c0, cn) in CG_C:
                            mm = nc.tensor.matmul(
                                ps[0:kr, c0:c0 + cn],
                                xts[cc][:, kc * 128:kc * 128 + kr],
                                qkvw[cc][:, 2 * C + c0:2 * C + c0 + cn],
                                start=(cc == 0),
                                stop=(cc == NCC - 1),
                            )
                            if cc == 0 and c0 == 0:
                                vfirsts.append(mm)
                    if kr < 128:
                        # zero the whole tile first (gpsimd memsets must be
                        # partition-aligned); the copies below fill 0:kr and
                        # rows kr:128 stay zero so the O matmul can stream
                        # the full 128-row stationary tile at 2.4 GHz
                        nc.gpsimd.memset(vp[kc][:], 0.0)
                    v3 = vp[kc][:].rearrange("p (h e) -> p h e", e=HD + 1)
                    nc.vector.tensor_copy(
                        v3[0:kr, :, 0:HD],
                        ps[0:kr, :].rearrange("p (h d) -> p h d", d=HD),
                    )
                    nc.vector.memset(v3[0:kr, :, HD:HD + 1], 1.0)
                return vp, vfirsts

            def alloc_oT(b):
                return [perb.tile([128, W], BF16, tag=f"oT{i}", name=f"oT{i}_{b}")
                        for i in range(NCC)]

            def emit_attn_pass1(b, j, qT, kTz):
                """S + exp + expB-multiply for head pair (2j, 2j+1).
                S matmuls use the full-128-row zero-padded kTz stationary so
                the PE runs at 2.4 GHz. Returns P tiles and a pacer matmul."""
                pts = [[None, None] for _ in range(NKC)]
                pacer = None
                for kc in range(NKC):
                    kr = _kr(kc)
                    ps_ss = [
                        psum_s.tile([128, W], F32, tag="s",
                                    name=f"pss{2 * j + hh}_{kc}_{b}")
                        for hh in range(2)
                    ]
                    for (c0, cn) in CG_N:
                        for hh in range(2):
                            h = 2 * j + hh
                            mm = nc.tensor.matmul(
                                ps_ss[hh][0:kr, c0:c0 + cn],
                                kTz[h][:, kc * 128:kc * 128 + kr],
                                qT[j][:, c0:c0 + cn],
                                start=True,
                                stop=True,
                            )
                            if kc == 2 and pacer is None:
                                pacer = mm
                    for hh in range(2):
                        h = 2 * j + hh
                        ebt = expbp.tile([128, W], BF16, tag="expb",
                                         name=f"ebt{h}_{kc}_{b}")
                        nc.vector.memset(ebt[:, N:W], 0.0)
                        nc.sync.dma_start(
                            ebt[0:kr, 0:N],
                            expB_d[h, kc * 128:kc * 128 + kr, :],
                        )
                        es = flow.tile([128, W], BF16, tag="expS",
                                       name=f"es{h}_{kc}_{b}")
                        nc.scalar.activation(
                            es[0:kr, 0:W], ps_ss[hh][0:kr, 0:W],
                            mybir.ActivationFunctionType.Exp,
                        )
                        pt = ptp.tile([128, W], BF16, tag="pT",
                                      name=f"pt{h}_{kc}_{b}")
                        if kr < 128:
                            # rows kr:128 must be zero (not stale garbage):
                            # the O matmul streams the full 128 rows and
                            # Inf/NaN garbage would poison psum even against
                            # zero V rows
                            nc.gpsimd.memset(pt[:], 0.0)
                        nc.vector.tensor_tensor(
                            pt[0:kr, 0:W],
                            es[0:kr, 0:W],
                            ebt[0:kr, 0:W],
                            mybir.AluOpType.mult,
                        )
                        pts[kc][hh] = pt
                return pts, pacer

            def emit_attn_pass2(b, j, pts, vp, oT, dall):
                """dense O-accumulation convoy for head pair (2j, 2j+1)."""
                for hh in range(2):
                    h = 2 * j + hh
                    ps_o = psum_s.tile([HD + 1, W], F32, tag="s",
                                       name=f"pso{h}_{b}")
                    for kc in range(NKC):
                        for (c0, cn) in CG_N:
                            nc.tensor.matmul(
                                ps_o[:, c0:c0 + cn],
                                vp[kc][0:128, h * (HD + 1):(h + 1) * (HD + 1)],
                                pts[kc][hh][0:128, c0:c0 + cn],
                                start=(kc == 0),
                                stop=(kc == NKC - 1),
                            )
                    nc.vector.tensor_copy(
                        oT[j][hh * 64:hh * 64 + 64, 0:N], ps_o[0:64, 0:N]
                    )
                    dn = norm.tile([65, W], BF16, tag="dn", bufs=1,
                                   name=f"dn{h}_{b}")
                    nc.vector.tensor_copy(dn[64:65, 0:N], ps_o[64:65, 0:N])
                    nc.sync.dma_start(dall[h:h + 1, 0:N], dn[64:65, 0:N])

            def emit_norm(b, oT, dall):
                # batched reciprocal + DMA broadcast + in-place normalize
                dinv = norm.tile([12, W], BF16, tag="dinv", name=f"dinv_{b}")
                with nc.allow_low_precision(
                        reason="bf16 softmax denominators: 0.1% rms, within budget"):
                    nc.vector.reciprocal(dinv[0:H, 0:N], dall[0:H, 0:N])
                nc.sync.dma_start(dinv_d[b], dinv[0:H, 0:N])
                for cc in range(NCC):
                    dr = norm.tile([128, W], BF16, tag="drep", bufs=2,
                                   name=f"dr{cc}_{b}")
                    for hh in range(2):
                        row = dinv_d[b, 2 * cc + hh, :]
                        srow = bass.AP(
                            tensor=row.tensor, offset=row.offset,
                            ap=[[0, 64]] + row.ap,
                        )
                        nc.sync.dma_start(dr[hh * 64:(hh + 1) * 64, 0:N], srow)
                    nc.vector.tensor_tensor(
                        oT[cc][:, 0:N], oT[cc][:, 0:N], dr[:, 0:N],
                        mybir.AluOpType.mult,
                    )
                return oT

            def emit_proj(b, oT):
                """transposed projection: out^T[co, n] = sum_ci pwT.T oT,
                bias fused on ACT via Identity activation."""
                pfirsts = []
                for co in range(NCC):
                    ps = psum_s.tile([128, W], F32, tag="s", name=f"psp{co}_{b}")
                    for cc in range(NCC):
                        for (c0, cn) in CG_N:
                            mm = nc.tensor.matmul(
                                ps[:, c0:c0 + cn],
                                pw16[cc][:, co * 128:(co + 1) * 128],
                                oT[cc][:, c0:c0 + cn],
                                start=(cc == 0),
                                stop=(cc == NCC - 1),
                            )
                            if cc == 0 and c0 == 0:
                                pfirsts.append(mm)
                    ob = outp.tile([128, W], F32, tag="ob", name=f"ob{co}_{b}")
                    nc.scalar.activation(
                        ob[:, 0:N], ps[:, 0:N],
                        mybir.ActivationFunctionType.Identity,
                        bias=pb_sb[:, co:co + 1],
                    )
                    nc.sync.dma_start(
                        out_d[b, co * 128:(co + 1) * 128, :], ob[:, 0:N]
                    )
                return pfirsts

            # software pipeline: batch 1's QKV work is emitted at lower
            # priority than batch 0's attention (and just-in-time between
            # batch 1's attention pairs) so the Tile scheduler uses it as PE
            # gap-filler during the exp/multiply-paced attention phases.
            # interleave weight/x chunk loads so the j=0 convoy's cc=0
            # matmul can start after the first two transfers instead of
            # waiting out the whole load train
            xts0 = []
            for cc in range(NCC):
                nc.sync.dma_start(
                    qkvw[cc][:], qkvwT_d[cc * 128:(cc + 1) * 128, :]
                )
                t = perb.tile([128, W], BF16, tag=f"xt{cc}", name=f"xt{cc}_0")
                nc.sync.dma_start(t[:], xT_d[0, cc * 128:(cc + 1) * 128, :])
                xts0.append(t)
            qT0 = [perb.tile([128, W], BF16, tag=f"qT{i}", name=f"qT{i}_0")
                   for i in range(NCC)]
            kTz0 = alloc_kTz(0)
            for j in range(NCC):
                emit_qkvT_chunk(0, xts0, j, qT0, kTz0, evac_vector=False)
            vp0, _ = emit_v(0, xts0)
            emit_weight_loads_proj()

            oT0 = alloc_oT(0)
            dall0 = norm.tile([12, W], BF16, tag="dall", bufs=1, name="dall_0")
            pacers0 = []
            pend0 = []
            for j in range(NCC):
                if j >= 1:
                    emit_attn_pass2(0, j - 1, pend0[j - 1], vp0, oT0, dall0)
                pts_j, pac = emit_attn_pass1(0, j, qT0, kTz0)
                pacers0.append(pac)
                pend0.append(pts_j)
            emit_attn_pass2(0, NCC - 1, pend0[NCC - 1], vp0, oT0, dall0)
            # fillers for batch-0 attention: x1 load + V1 + qkvT1, paced so
            # the greedy scheduler doesn't front-load them all at once
            xts1 = emit_x(1)
            vp1, vfirsts1 = emit_v(1, xts1)
            for kc, f in enumerate(vfirsts1):
                add_dep_helper(f.ins, pacers0[min(kc, NCC - 1)].ins, sync=False,
                               reason="pace v1 filler")
            emit_norm(0, oT0, dall0)

            qT1 = [perb.tile([128, W], BF16, tag=f"qT{i}", name=f"qT{i}_1")
                   for i in range(NCC)]
            kTz1 = alloc_kTz(1)
            oT1 = alloc_oT(1)
            dall1 = norm.tile([12, W], BF16, tag="dall", bufs=1, name="dall_1")
            pacers1 = []
            pend1 = []
            for j in range(NCC):
                # just-in-time qkv chunk for pair j, used as gap-filler.
                qf = emit_qkvT_chunk(1, xts1, j, qT1, kTz1, evac_vector=True)
                pace = pacers0[j + 4] if j < 2 else pacers1[j - 2]
                for f in qf:
                    add_dep_helper(f.ins, pace.ins, sync=False,
                                   reason="pace qkvT1 filler")
                if j >= 1:
                    emit_attn_pass2(1, j - 1, pend1[j - 1], vp1, oT1, dall1)
                pts_j, pac = emit_attn_pass1(1, j, qT1, kTz1)
                pacers1.append(pac)
                pend1.append(pts_j)
            emit_attn_pass2(1, NCC - 1, pend1[NCC - 1], vp1, oT1, dall1)
            # proj0: paced across batch-1 attention pairs as its PE filler
            pfirsts0 = emit_proj(0, oT0)
            for tt, f in enumerate(pfirsts0):
                add_dep_helper(f.ins, pacers1[min(tt, NCC - 1)].ins, sync=False,
                               reason="pace proj0 filler")
            emit_norm(1, oT1, dall1)
            emit_proj(1, oT1)

    nc.compile()
    return nc


def _relative_position_index():
    coords = np.stack(np.meshgrid(np.arange(WX), np.arange(WY), indexing="ij"))
    cf = coords.reshape(2, -1)
    rel = cf[:, :, None] - cf[:, None, :]
    rel = rel.transpose(1, 2, 0).astype(np.int64)
    rel[:, :, 0] += WX - 1
    rel[:, :, 1] += WY - 1
    rel[:, :, 0] *= 2 * WY - 1
    return rel.sum(-1)  # [L, L]


def _host_prep(x, qkv_w, proj_w, proj_b, rel_table, g2l, g2g):
    x = np.asarray(x, np.float32)
    qkv_w = np.asarray(qkv_w, np.float32)
    proj_w = np.asarray(proj_w, np.float32)
    proj_b = np.asarray(proj_b, np.float32)
    rel_table = np.asarray(rel_table, np.float32)
    g2l = np.asarray(g2l, np.float32)
    g2g = np.asarray(g2g, np.float32)

    bf16 = ml_dtypes.bfloat16
    xT = np.zeros((B, C, W), np.float32)
    xT[:, :, :N] = x.transpose(0, 2, 1)
    xT = xT.astype(bf16)
    qkv_wT = np.ascontiguousarray(qkv_w.T).copy()                  # [C, 3C]
    qkv_wT[:, :C] *= SCALE                                         # fold q scale
    qkv_wT = qkv_wT.astype(bf16)
    proj_wT = np.ascontiguousarray(proj_w.T).astype(bf16)          # [C, C]
    pb = np.ascontiguousarray(proj_b.reshape(NCC, 128).T)          # [128, NCC]

    # expB[h, k, q] = exp(bias[h, q, k]); exp applied at table granularity,
    # then expanded by the constant-index relative-position gather.
    ridx = _relative_position_index()
    et = np.exp(rel_table)                                         # [3025, H]
    eg2l = np.exp(g2l)                                             # [2, H, 1]
    eg2g = np.exp(g2g)                                             # [H, 1, 1]
    expB = np.empty((H, N, N), np.float32)
    expB[:, 1:, 1:] = et[ridx].transpose(2, 1, 0)                  # [H, k, q]
    expB[:, 0, 0] = eg2g[:, 0, 0]
    expB[:, 1:, 0] = eg2l[0][:, 0][None, :].T                      # global query
    expB[:, 0, 1:] = eg2l[1][:, 0][:, None]                        # global key
    expB16 = expB.astype(bf16)

    in_maps = []
    for i in range(N_CORES):
        in_maps.append({
            "xT": xT[i * B_LOC:(i + 1) * B_LOC],
            "qkv_wT": qkv_wT,
            "proj_wT": proj_wT,
            "proj_b": pb,
            "expB": expB16,
        })
    return in_maps


_NC = None


def get_nc():
    global _NC
    if _NC is None:
        _NC = build_nc()
    return _NC


def kernel(x, qkv_w, proj_w, proj_b, rel_table, g2l, g2g):
    in_maps = _host_prep(x, qkv_w, proj_w, proj_b, rel_table, g2l, g2g)
    nc = get_nc()
    res = run_bass_kernel_spmd(nc, in_maps, core_ids=list(range(N_CORES)))
    # device emits out^T [B_LOC, C, N]; un-transpose on host
    out = np.concatenate(
        [res.results[i]["out"].transpose(0, 2, 1) for i in range(N_CORES)],
        axis=0,
    )
    return np.ascontiguousarray(out).astype(np.float32)
